# revision 33
# baseline (speedup 1.0000x reference)
"""MoE top-2 -> per-expert Linear -> gated combine, SINGLE NEFF per core.

Data-parallel over tokens (~2048/core: both pairs of a token live on its
core).  Tokens are dealt ROUND-ROBIN WITHIN each ordered expert-combo
(e1, e2) group, and every combo run is padded to the shared per-core max
m_ij = ceil(n_ij / 8), so all 8 cores share ONE program (same segment
lengths, same combo runs; dummy columns carry gate 0).

Pool layout: per-expert segments [A-pairs | B-pairs], both in (e1,e2,tok)
order -> every ordered combo occupies CONTIGUOUS runs in segment e1's
A-block, segment e2's B-block, and the output columns.  The combine is 56
contiguous DVE adds out of an SBUF-resident transposed pool — no gather,
no second NEFF, no DRAM round-trip for y.

Compute is W-stationary in the transposed domain: psum[oc*128, n] +=
W_e[ko, oc].T @ xT[ko, cols], accumulated over ko, evicted fp32->fp16
into the pool (vector/scalar engines alternate).  Gates folded into xT
host-side.  Output out^T [128, 8, BT_eff] fp16; host unpacks.
"""

import os
import sys
import types

sys.path.insert(0, "/opt/trn_rl_repo")

import ml_dtypes
import numpy as np

import concourse.bass as bass
import concourse.mybir as mybir
from concourse import bass_utils
from concourse.tile import TileContext

B, E, D, O = 16384, 8, 1024, 1024
N_CORES = 8
P = 128
KO = D // P
OC = O // P
CHMAX = 512
MAX_WAITS = int(os.environ.get("MOE_MAX_WAITS", "1"))

_DT_MAP = {
    "float16": (mybir.dt.float16, np.float16),
    "bfloat16": (mybir.dt.bfloat16, ml_dtypes.bfloat16),
}

def _patch_tile_drain():
    """Public-walrus workaround: walrus codegen rejects instructions carrying
    more than a couple of sync-wait commands.  Tile's add_semaphores can put
    several waits on one instruction (and the kernel-tail drain carries one
    per live processor).  Hoist excess waits onto single-wait nop carriers
    emitted just before the instruction on the same engine."""
    from concourse.tile import TileContext as TC
    from concourse.vector_clock import ScopedClock

    if getattr(TC, "_moe_drain_patched", False):
        return

    orig_add = TC._add_instruction

    def _add_instruction(self, inst):
        si = getattr(inst, "sync_info", None)
        waits = list(si.on_wait or []) if si is not None else []
        if len(waits) > MAX_WAITS:
            hoist = waits[: len(waits) - MAX_WAITS]
            keep = waits[len(waits) - MAX_WAITS :]
            for w in hoist:
                nop = mybir.InstNoOp(
                    name=self.nc.get_next_instruction_name(),
                    engine=inst.engine,
                    bass_nofuse=True,
                    sync_info=mybir.SyncInfo(on_wait=[w], on_update=[]),
                )
                orig_add(self, nop)
            inst.sync_info = mybir.SyncInfo(
                on_wait=keep, on_update=list(si.on_update or [])
            )
        orig_add(self, inst)

    def _drain_and_barrier(self, tick_clock, wait_clock):
        carrier = self.nc.sync.nop(nofuse=True)
        wait_clock.add_sem_waits(
            carrier.ins, ScopedClock({None: tick_clock.global_clock})
        )
        si = carrier.ins.sync_info
        waits = list(si.on_wait or []) if si is not None else []
        if len(waits) > 1:
            carrier.ins.sync_info = mybir.SyncInfo(
                on_wait=waits[:1], on_update=list(si.on_update or [])
            )
            for w in waits[1:]:
                extra = self.nc.sync.nop(nofuse=True)
                extra.ins.sync_info = mybir.SyncInfo(on_wait=[w], on_update=[])
        self.nc.sync.drain()
        self.nc.all_engine_barrier()
        assert self.sems is not None
        popped = self.nc._tile_sem_poison_stack.pop()
        assert popped is self._sem_poison
        self.nc.clear_and_free_semaphores(list(self.sems.allocated().values()))
        self.nc.all_engine_barrier()

    TC._add_instruction = _add_instruction
    TC._drain_and_barrier = _drain_and_barrier
    TC._moe_drain_patched = True




def _install_ntff_shim():
    """Best-effort: register the missing antenv.axon_hooks NTFF profile hook
    so trace=True yields exec_time_ns.  Only used when MOE_TRACE=1."""
    try:
        import antenv
        from trn_agent_boot.trn_boot import _ntff_profile_via_ctypes

        if "antenv.axon_hooks" in sys.modules:
            return
        hooks = types.ModuleType("antenv.axon_hooks")
        hook = _ntff_profile_via_ctypes("/opt/axon/libaxon_pjrt.so")
        hooks.get_axon_ntff_profile_hook = lambda: hook
        hooks.set_axon_ntff_profile_hook = lambda h: None
        sys.modules["antenv.axon_hooks"] = hooks
        antenv.axon_hooks = hooks
        bass_utils.upload_artifacts = lambda tmpdir: tmpdir
    except Exception as e:  # pragma: no cover
        print(f"ntff shim unavailable: {e}", file=sys.stderr)




def _plan(gates):
    """Shared schedule + per-core token fills.

    Returns (sched, fills):
      sched: m[E,E] padded combo counts, seg_len/seg_off[E], combos list
             (lex order) with runs, BT_eff, NPAIR_eff
      fills[c]: dict(tok -> [BT_eff] global token id or -1 (dummy),
                g1, g2 -> gates for A/B pair of each column-slot)
    """
    exp = np.argsort(-gates, axis=1)[:, :2]
    e1, e2 = exp[:, 0], exp[:, 1]
    order = np.lexsort((np.arange(B), e2, e1))
    se1, se2 = e1[order], e2[order]
    n = np.zeros((E, E), np.int64)
    np.add.at(n, (se1, se2), 1)
    m = 2 * (-(-n // (N_CORES * 2)))  # ceil to multiple of 2 (4B AP alignment)
    BT_eff = int(m.sum())
    NPAIR_eff = 2 * BT_eff

    a_len = m.sum(1)
    b_len = m.sum(0)
    seg_len_e = a_len + b_len  # per EXPERT
    # processing order: largest expert first (more columns during the
    # W-starved startup), smallest last (fewer tail combine columns)
    eorder = np.argsort(-seg_len_e, kind="stable")
    posof = np.empty(E, np.int64)
    posof[eorder] = np.arange(E)
    seg_len = seg_len_e[eorder]  # by position
    seg_off_p = np.concatenate([[0], np.cumsum(seg_len)[:-1]])
    seg_off_e = seg_off_p[posof]  # by expert id
    a_run = np.zeros((E, E), np.int64)
    b_run = np.zeros((E, E), np.int64)
    for i in range(E):
        a_run[i] = seg_off_e[i] + np.concatenate([[0], np.cumsum(m[i])[:-1]])
    for j in range(E):
        b_run[:, j] = (
            seg_off_e[j] + a_len[j] + np.concatenate([[0], np.cumsum(m[:, j])[:-1]])
        )
    combos = []
    pos = 0
    for i in range(E):
        for j in range(E):
            cnt = int(n[i, j])
            toks = order[pos : pos + cnt]
            pos += cnt
            if m[i, j]:
                combos.append(dict(i=i, j=j, n=int(m[i, j]), a=int(a_run[i, j]),
                                   b=int(b_run[i, j]),
                                   ready=int(max(posof[i], posof[j])),
                                   toks=toks))
    assert pos == B
    # OUT columns in readiness order: stores then cover a contiguous,
    # monotonically growing prefix (no store can snapshot unwritten cols)
    combos.sort(key=lambda cb: (cb["ready"], cb["i"], cb["j"]))
    o = 0
    for cb in combos:
        cb["o"] = o
        o += cb["n"]
    assert o == BT_eff
    sched = dict(m=m, n=n, seg_len=seg_len, seg_off=seg_off_p, combos=combos,
                 BT_eff=BT_eff, NPAIR_eff=NPAIR_eff, eorder=eorder,
                 a_run=a_run, b_run=b_run)

    # deal each combo's tokens round-robin to cores
    fills = [dict(tok=np.full(BT_eff, -1, np.int64),
                  g1=np.zeros(BT_eff, np.float32),
                  g2=np.zeros(BT_eff, np.float32)) for _ in range(N_CORES)]
    for cb in combos:
        i, j, o0 = cb["i"], cb["j"], cb["o"]
        for c in range(N_CORES):
            mine = cb["toks"][c::N_CORES]
            k = len(mine)
            fills[c]["tok"][o0 : o0 + k] = mine
            fills[c]["g1"][o0 : o0 + k] = gates[mine, i]
            fills[c]["g2"][o0 : o0 + k] = gates[mine, j]
    return sched, fills


def _core_inputs(x, sched, fill, np_dt):
    BT_eff, NPAIR_eff = sched["BT_eff"], sched["NPAIR_eff"]
    tok = fill["tok"]
    safe_tok = np.where(tok >= 0, tok, 0)
    cols_tok = np.zeros(NPAIR_eff, np.int64)
    cols_g = np.zeros(NPAIR_eff, np.float32)
    # column of A pair of slot s = a_run equivalent: slots map 1:1 by combo
    a_cols = np.zeros(BT_eff, np.int64)
    b_cols = np.zeros(BT_eff, np.int64)
    for cb in sched["combos"]:
        s = slice(cb["o"], cb["o"] + cb["n"])
        a_cols[s] = np.arange(cb["a"], cb["a"] + cb["n"])
        b_cols[s] = np.arange(cb["b"], cb["b"] + cb["n"])
    cols_tok[a_cols] = safe_tok
    cols_tok[b_cols] = safe_tok
    cols_g[a_cols] = fill["g1"]
    cols_g[b_cols] = fill["g2"]
    xg = x[cols_tok] * cols_g[:, None]
    xt = xg.astype(np_dt).reshape(NPAIR_eff, KO, P).transpose(2, 1, 0).copy()
    return {"xt": xt}


def _chunks(L):
    # near-equal pieces <= CHMAX, 4-aligned: avoids tiny-N matmuls
    k = -(-L // CHMAX)
    out, l0 = [], 0
    for i in range(k):
        nn_ = (L - l0) // (k - i)
        nn_ = min(L - l0, -(-nn_ // 4) * 4)
        out.append((l0, nn_))
        l0 += nn_
    assert l0 == L
    return out


def _build_program(sched, dt, ydt):
    NPAIR_eff, BT_eff = sched["NPAIR_eff"], sched["BT_eff"]
    dbg = bool(os.environ.get("MOE_DEBUG_POOL"))
    nc = bass.Bass(target_bir_lowering=False, trn_type="TRN2")
    xt_d = nc.dram_tensor("xt", [P, KO, NPAIR_eff], dt, kind="ExternalInput")
    w_d = nc.dram_tensor("w", [E, P, KO, O], dt, kind="ExternalInput")
    out_d = nc.dram_tensor("out", [P, OC, BT_eff], ydt, kind="ExternalOutput")
    if dbg:
        pool_d = nc.dram_tensor("pooldbg", [P, OC, NPAIR_eff], ydt,
                                kind="ExternalOutput")

    seg_len, seg_off, combos = sched["seg_len"], sched["seg_off"], sched["combos"]
    # store bounds: three quarters, then the last quarter in two halves so
    # the final (post-last-matmul) store is small and overlaps the adds
    q = BT_eff // 4
    bound = [q, 2 * q, 3 * q, (3 * q + BT_eff) // 2, BT_eff]
    NSTORE = len(bound)

    with TileContext(nc) as tc:
        with (
            tc.tile_pool(name="pool", bufs=1) as ppool,
            tc.tile_pool(name="wp", bufs=2) as wpool,
            tc.tile_pool(name="xc", bufs=5) as xpool,
            tc.tile_pool(name="xl", bufs=1) as lpool,
            tc.tile_pool(name="ob", bufs=1) as opool,
            tc.tile_pool(name="ps", bufs=8, space="PSUM") as pspool,
        ):
            pool = ppool.tile([P, OC, NPAIR_eff], ydt)
            out_sb = opool.tile([P, OC, BT_eff], ydt)
            emitted = [False] * NSTORE

            def emit_stores(done_cols):
                for st in range(NSTORE):
                    if not emitted[st] and done_cols >= bound[st]:
                        lo = bound[st - 1] if st else 0
                        sl = slice(lo, bound[st])
                        nc.sync.dma_start(
                            out=out_d[:, :, sl], in_=out_sb[:, :, sl]
                        )
                        emitted[st] = True

            eorder = sched["eorder"]
            for p in range(E):
                e_src = int(eorder[p])
                L, off = int(seg_len[p]), int(seg_off[p])
                # one tile PER ko chunk: Tile deps are tile-granular, so a
                # matmul only waits for the chunk it actually reads
                w_k = [wpool.tile([P, O], dt, tag=f"w{h}", name=f"wk{h}")
                       for h in range(KO)]

                def emit_w(h, eng):
                    eng.dma_start(out=w_k[h][:], in_=w_d[e_src, :, h, :])

                if p == 0:  # small lead chunk: first matmuls start sooner
                    seg_chunks = [(0, 256)] + [
                        (256 + l0, nn_) for l0, nn_ in _chunks(L - 256)
                    ]
                    # ramp-critical emission, balanced 8+8 across BOTH
                    # HWDGE queues (issue ~0.65us/op each): together they
                    # deliver one full (W, xl) ko-pair per 0.65us, in ko
                    # order, which leads the ko-outer sweep's consumption
                    # (one ko-pair per ~0.44us starting only at +10.8)
                    ln = seg_chunks[0][1]
                    xl = [lpool.tile([P, 256], dt, tag=f"xl{h}", name=f"xl{h}")
                          for h in range(KO)]

                    def emit_xl(h, eng):
                        eng.dma_start(
                            out=xl[h][:, :ln], in_=xt_d[:, h, off : off + ln]
                        )

                    for h in range(KO):
                        if h % 2 == 0:
                            emit_w(h, nc.scalar)
                            emit_xl(h, nc.sync)
                        else:
                            emit_w(h, nc.sync)
                            emit_xl(h, nc.scalar)
                else:
                    seg_chunks = _chunks(L)
                    for h in range(KO):
                        emit_w(h, nc.scalar if h % 2 == 0 else nc.sync)
                for ci, (l0, nn_) in enumerate(seg_chunks):
                    xcf = xpool.tile([P, KO, CHMAX], dt, tag="x")
                    xc = xcf[:, :, :nn_]
                    if p == 0 and ci == 0:
                        pass  # lead xc already emitted above (xl tiles)
                    else:
                        nc.sync.dma_start(
                            out=xc, in_=xt_d[:, :, off + l0 : off + l0 + nn_]
                        )
                    use_xl = p == 0 and ci == 0

                    def evict(oc, ps):
                        if oc % 2 == 0 or os.environ.get("MOE_NO_ACT"):
                            nc.vector.tensor_copy(
                                out=pool[:, oc, off + l0 : off + l0 + nn_],
                                in_=ps,
                            )
                        else:
                            nc.scalar.copy(
                                out=pool[:, oc, off + l0 : off + l0 + nn_],
                                in_=ps,
                            )

                    if p == 0:
                        # W arrives chunk-by-chunk at DMA-issue rate: run
                        # ko OUTER over half the oc banks so each arriving
                        # W chunk feeds 4 matmuls (consumption matches
                        # delivery); 4 PSUM banks per half keep eviction
                        # of one half under the other half's matmuls.
                        for half in range(2):
                            pss = []
                            for oc in range(half * 4, half * 4 + 4):
                                psf = pspool.tile(
                                    [P, CHMAX], mybir.dt.float32, tag="ps"
                                )
                                pss.append(psf[:, :nn_])
                            for ko in range(KO):
                                for k, oc in enumerate(
                                    range(half * 4, half * 4 + 4)
                                ):
                                    nc.tensor.matmul(
                                        out=pss[k],
                                        lhsT=w_k[ko][:, oc * P : (oc + 1) * P],
                                        rhs=(xl[ko][:, :nn_]
                                             if use_xl
                                             else xc[:, ko, :]),
                                        start=(ko == 0),
                                        stop=(ko == KO - 1),
                                    )
                            for k, oc in enumerate(range(half * 4, half * 4 + 4)):
                                evict(oc, pss[k])
                    else:
                        for oc in range(OC):
                            psf = pspool.tile(
                                [P, CHMAX], mybir.dt.float32, tag="ps"
                            )
                            ps = psf[:, :nn_]
                            for ko in range(KO):
                                nc.tensor.matmul(
                                    out=ps,
                                    lhsT=w_k[ko][:, oc * P : (oc + 1) * P],
                                    rhs=xc[:, ko, :],
                                    start=(ko == 0),
                                    stop=(ko == KO - 1),
                                )
                            evict(oc, ps)
                if p == E - 1:
                    # final group: oc-half adds, all low-half ops first —
                    # their evictions (oc 0-3, emitted before oc 4-7)
                    # complete while the high-oc matmuls still run, so the
                    # whole low pass hides under them (DVE is FIFO)
                    for h0 in (0, 4):
                        for cb in combos:
                            if cb["ready"] != p:
                                continue
                            nc.vector.tensor_add(
                                out=out_sb[:, h0 : h0 + 4,
                                           cb["o"] : cb["o"] + cb["n"]],
                                in0=pool[:, h0 : h0 + 4,
                                         cb["a"] : cb["a"] + cb["n"]],
                                in1=pool[:, h0 : h0 + 4,
                                         cb["b"] : cb["b"] + cb["n"]],
                            )
                            if h0 == 4:
                                emit_stores(cb["o"] + cb["n"])
                else:
                    for cb in combos:
                        if cb["ready"] == p:
                            nc.vector.tensor_add(
                                out=out_sb[:, :, cb["o"] : cb["o"] + cb["n"]],
                                in0=pool[:, :, cb["a"] : cb["a"] + cb["n"]],
                                in1=pool[:, :, cb["b"] : cb["b"] + cb["n"]],
                            )
                done = sum(cb["n"] for cb in combos if cb["ready"] <= p)
                emit_stores(done)
            assert all(emitted)
            if dbg:
                nc.sync.dma_start(out=pool_d[:, :, :], in_=pool[:])
    return nc


def kernel(x, gates, W, b):
    _patch_tile_drain()
    dt, np_dt = _DT_MAP[os.environ.get("MOE_DT", "float16")]
    ydt, _ = _DT_MAP[os.environ.get("MOE_YDT", "float16")]

    gates = np.asarray(gates)
    x = np.ascontiguousarray(x)
    W = np.asarray(W)
    b = np.asarray(b)
    assert not np.any(b != 0)

    sched, fills = _plan(gates)
    nc = _build_program(sched, dt, ydt)
    wb = W.astype(np_dt).reshape(E, KO, P, O).transpose(0, 2, 1, 3).copy()
    in_maps = []
    for c in range(N_CORES):
        im = _core_inputs(x, sched, fills[c], np_dt)
        im["w"] = wb
        in_maps.append(im)

    trace = os.environ.get("MOE_TRACE", "0") == "1"
    kwargs = {}
    if trace:
        _install_ntff_shim()
        kwargs = dict(trace=True, trace_cores=list(range(N_CORES)))

    res = bass_utils.run_bass_kernel_spmd(
        nc, in_maps, core_ids=list(range(N_CORES)), **kwargs
    )
    if trace and res.exec_time_ns is not None:
        print(f"HW exec time: {res.exec_time_ns} ns "
              f"(single NEFF; mean {res.mean_exec_time_ns:.0f})")
    out = np.empty((B, O), np.float32)
    for c in range(N_CORES):
        ot = res.results[c]["out"]  # [P, OC, BT_eff]
        rows = ot.transpose(2, 1, 0).reshape(sched["BT_eff"], O)
        tok = fills[c]["tok"]
        real = tok >= 0
        out[tok[real]] = rows[real].astype(np.float32)
    return out


# revision 34
# speedup vs baseline: 1.0114x; 1.0114x over previous
"""MoE top-2 -> per-expert Linear -> gated combine, SINGLE NEFF per core.

Data-parallel over tokens (~2048/core: both pairs of a token live on its
core).  Tokens are dealt ROUND-ROBIN WITHIN each ordered expert-combo
(e1, e2) group, and every combo run is padded to the shared per-core max
m_ij = ceil(n_ij / 8), so all 8 cores share ONE program (same segment
lengths, same combo runs; dummy columns carry gate 0).

Pool layout: per-expert segments [A-pairs | B-pairs], both in (e1,e2,tok)
order -> every ordered combo occupies CONTIGUOUS runs in segment e1's
A-block, segment e2's B-block, and the output columns.  The combine is 56
contiguous DVE adds out of an SBUF-resident transposed pool — no gather,
no second NEFF, no DRAM round-trip for y.

Compute is W-stationary in the transposed domain: psum[oc*128, n] +=
W_e[ko, oc].T @ xT[ko, cols], accumulated over ko, evicted fp32->fp16
into the pool (vector/scalar engines alternate).  Gates folded into xT
host-side.  Output out^T [128, 8, BT_eff] fp16; host unpacks.
"""

import os
import sys
import types

sys.path.insert(0, "/opt/trn_rl_repo")

import ml_dtypes
import numpy as np

import concourse.bass as bass
import concourse.mybir as mybir
from concourse import bass_utils
from concourse.tile import TileContext

B, E, D, O = 16384, 8, 1024, 1024
N_CORES = 8
P = 128
KO = D // P
OC = O // P
CHMAX = 512
MAX_WAITS = int(os.environ.get("MOE_MAX_WAITS", "1"))

_DT_MAP = {
    "float16": (mybir.dt.float16, np.float16),
    "bfloat16": (mybir.dt.bfloat16, ml_dtypes.bfloat16),
}

def _patch_tile_drain():
    """Public-walrus workaround: walrus codegen rejects instructions carrying
    more than a couple of sync-wait commands.  Tile's add_semaphores can put
    several waits on one instruction (and the kernel-tail drain carries one
    per live processor).  Hoist excess waits onto single-wait nop carriers
    emitted just before the instruction on the same engine."""
    from concourse.tile import TileContext as TC
    from concourse.vector_clock import ScopedClock

    if getattr(TC, "_moe_drain_patched", False):
        return

    orig_add = TC._add_instruction

    def _add_instruction(self, inst):
        si = getattr(inst, "sync_info", None)
        waits = list(si.on_wait or []) if si is not None else []
        if len(waits) > MAX_WAITS:
            hoist = waits[: len(waits) - MAX_WAITS]
            keep = waits[len(waits) - MAX_WAITS :]
            for w in hoist:
                nop = mybir.InstNoOp(
                    name=self.nc.get_next_instruction_name(),
                    engine=inst.engine,
                    bass_nofuse=True,
                    sync_info=mybir.SyncInfo(on_wait=[w], on_update=[]),
                )
                orig_add(self, nop)
            inst.sync_info = mybir.SyncInfo(
                on_wait=keep, on_update=list(si.on_update or [])
            )
        orig_add(self, inst)

    def _drain_and_barrier(self, tick_clock, wait_clock):
        carrier = self.nc.sync.nop(nofuse=True)
        wait_clock.add_sem_waits(
            carrier.ins, ScopedClock({None: tick_clock.global_clock})
        )
        si = carrier.ins.sync_info
        waits = list(si.on_wait or []) if si is not None else []
        if len(waits) > 1:
            carrier.ins.sync_info = mybir.SyncInfo(
                on_wait=waits[:1], on_update=list(si.on_update or [])
            )
            for w in waits[1:]:
                extra = self.nc.sync.nop(nofuse=True)
                extra.ins.sync_info = mybir.SyncInfo(on_wait=[w], on_update=[])
        self.nc.sync.drain()
        self.nc.all_engine_barrier()
        assert self.sems is not None
        popped = self.nc._tile_sem_poison_stack.pop()
        assert popped is self._sem_poison
        if os.environ.get("MOE_FULL_EXIT"):
            self.nc.clear_and_free_semaphores(
                list(self.sems.allocated().values())
            )
            self.nc.all_engine_barrier()
        else:
            # skip the exit-time semaphore-clear storm (~5us of serialized
            # sem writes after all real work): every execution's preamble
            # re-initializes the semaphores it uses, so the device-side
            # clears are redundant at end-of-program.  Host bookkeeping
            # (free list/poison) is irrelevant here — nothing is emitted
            # after this point.
            pass

    TC._add_instruction = _add_instruction
    TC._drain_and_barrier = _drain_and_barrier
    TC._moe_drain_patched = True




def _install_ntff_shim():
    """Best-effort: register the missing antenv.axon_hooks NTFF profile hook
    so trace=True yields exec_time_ns.  Only used when MOE_TRACE=1."""
    try:
        import antenv
        from trn_agent_boot.trn_boot import _ntff_profile_via_ctypes

        if "antenv.axon_hooks" in sys.modules:
            return
        hooks = types.ModuleType("antenv.axon_hooks")
        hook = _ntff_profile_via_ctypes("/opt/axon/libaxon_pjrt.so")
        hooks.get_axon_ntff_profile_hook = lambda: hook
        hooks.set_axon_ntff_profile_hook = lambda h: None
        sys.modules["antenv.axon_hooks"] = hooks
        antenv.axon_hooks = hooks
        bass_utils.upload_artifacts = lambda tmpdir: tmpdir
    except Exception as e:  # pragma: no cover
        print(f"ntff shim unavailable: {e}", file=sys.stderr)




def _plan(gates):
    """Shared schedule + per-core token fills.

    Returns (sched, fills):
      sched: m[E,E] padded combo counts, seg_len/seg_off[E], combos list
             (lex order) with runs, BT_eff, NPAIR_eff
      fills[c]: dict(tok -> [BT_eff] global token id or -1 (dummy),
                g1, g2 -> gates for A/B pair of each column-slot)
    """
    exp = np.argsort(-gates, axis=1)[:, :2]
    e1, e2 = exp[:, 0], exp[:, 1]
    order = np.lexsort((np.arange(B), e2, e1))
    se1, se2 = e1[order], e2[order]
    n = np.zeros((E, E), np.int64)
    np.add.at(n, (se1, se2), 1)
    m = 2 * (-(-n // (N_CORES * 2)))  # ceil to multiple of 2 (4B AP alignment)
    BT_eff = int(m.sum())
    NPAIR_eff = 2 * BT_eff

    a_len = m.sum(1)
    b_len = m.sum(0)
    seg_len_e = a_len + b_len  # per EXPERT
    # processing order: largest expert first (more columns during the
    # W-starved startup), smallest last (fewer tail combine columns)
    eorder = np.argsort(-seg_len_e, kind="stable")
    posof = np.empty(E, np.int64)
    posof[eorder] = np.arange(E)
    seg_len = seg_len_e[eorder]  # by position
    seg_off_p = np.concatenate([[0], np.cumsum(seg_len)[:-1]])
    seg_off_e = seg_off_p[posof]  # by expert id
    a_run = np.zeros((E, E), np.int64)
    b_run = np.zeros((E, E), np.int64)
    for i in range(E):
        a_run[i] = seg_off_e[i] + np.concatenate([[0], np.cumsum(m[i])[:-1]])
    for j in range(E):
        b_run[:, j] = (
            seg_off_e[j] + a_len[j] + np.concatenate([[0], np.cumsum(m[:, j])[:-1]])
        )
    combos = []
    pos = 0
    for i in range(E):
        for j in range(E):
            cnt = int(n[i, j])
            toks = order[pos : pos + cnt]
            pos += cnt
            if m[i, j]:
                combos.append(dict(i=i, j=j, n=int(m[i, j]), a=int(a_run[i, j]),
                                   b=int(b_run[i, j]),
                                   ready=int(max(posof[i], posof[j])),
                                   toks=toks))
    assert pos == B
    # OUT columns in readiness order: stores then cover a contiguous,
    # monotonically growing prefix (no store can snapshot unwritten cols)
    combos.sort(key=lambda cb: (cb["ready"], cb["i"], cb["j"]))
    o = 0
    for cb in combos:
        cb["o"] = o
        o += cb["n"]
    assert o == BT_eff
    sched = dict(m=m, n=n, seg_len=seg_len, seg_off=seg_off_p, combos=combos,
                 BT_eff=BT_eff, NPAIR_eff=NPAIR_eff, eorder=eorder,
                 a_run=a_run, b_run=b_run)

    # deal each combo's tokens round-robin to cores
    fills = [dict(tok=np.full(BT_eff, -1, np.int64),
                  g1=np.zeros(BT_eff, np.float32),
                  g2=np.zeros(BT_eff, np.float32)) for _ in range(N_CORES)]
    for cb in combos:
        i, j, o0 = cb["i"], cb["j"], cb["o"]
        for c in range(N_CORES):
            mine = cb["toks"][c::N_CORES]
            k = len(mine)
            fills[c]["tok"][o0 : o0 + k] = mine
            fills[c]["g1"][o0 : o0 + k] = gates[mine, i]
            fills[c]["g2"][o0 : o0 + k] = gates[mine, j]
    return sched, fills


def _core_inputs(x, sched, fill, np_dt):
    BT_eff, NPAIR_eff = sched["BT_eff"], sched["NPAIR_eff"]
    tok = fill["tok"]
    safe_tok = np.where(tok >= 0, tok, 0)
    cols_tok = np.zeros(NPAIR_eff, np.int64)
    cols_g = np.zeros(NPAIR_eff, np.float32)
    # column of A pair of slot s = a_run equivalent: slots map 1:1 by combo
    a_cols = np.zeros(BT_eff, np.int64)
    b_cols = np.zeros(BT_eff, np.int64)
    for cb in sched["combos"]:
        s = slice(cb["o"], cb["o"] + cb["n"])
        a_cols[s] = np.arange(cb["a"], cb["a"] + cb["n"])
        b_cols[s] = np.arange(cb["b"], cb["b"] + cb["n"])
    cols_tok[a_cols] = safe_tok
    cols_tok[b_cols] = safe_tok
    cols_g[a_cols] = fill["g1"]
    cols_g[b_cols] = fill["g2"]
    xg = x[cols_tok] * cols_g[:, None]
    xt = xg.astype(np_dt).reshape(NPAIR_eff, KO, P).transpose(2, 1, 0).copy()
    return {"xt": xt}


def _chunks(L):
    # near-equal pieces <= CHMAX, 4-aligned: avoids tiny-N matmuls
    k = -(-L // CHMAX)
    out, l0 = [], 0
    for i in range(k):
        nn_ = (L - l0) // (k - i)
        nn_ = min(L - l0, -(-nn_ // 4) * 4)
        out.append((l0, nn_))
        l0 += nn_
    assert l0 == L
    return out


def _build_program(sched, dt, ydt):
    NPAIR_eff, BT_eff = sched["NPAIR_eff"], sched["BT_eff"]
    dbg = bool(os.environ.get("MOE_DEBUG_POOL"))
    nc = bass.Bass(target_bir_lowering=False, trn_type="TRN2")
    xt_d = nc.dram_tensor("xt", [P, KO, NPAIR_eff], dt, kind="ExternalInput")
    w_d = nc.dram_tensor("w", [E, P, KO, O], dt, kind="ExternalInput")
    out_d = nc.dram_tensor("out", [P, OC, BT_eff], ydt, kind="ExternalOutput")
    if dbg:
        pool_d = nc.dram_tensor("pooldbg", [P, OC, NPAIR_eff], ydt,
                                kind="ExternalOutput")

    seg_len, seg_off, combos = sched["seg_len"], sched["seg_off"], sched["combos"]
    # store bounds: three quarters, then the last quarter in two halves so
    # the final (post-last-matmul) store is small and overlaps the adds
    q = BT_eff // 4
    bound = [q, 2 * q, 3 * q, (3 * q + BT_eff) // 2, BT_eff]
    NSTORE = len(bound)

    with TileContext(nc) as tc:
        with (
            tc.tile_pool(name="pool", bufs=1) as ppool,
            tc.tile_pool(name="wp", bufs=2) as wpool,
            tc.tile_pool(name="xc", bufs=5) as xpool,
            tc.tile_pool(name="xl", bufs=1) as lpool,
            tc.tile_pool(name="ob", bufs=1) as opool,
            tc.tile_pool(name="ps", bufs=8, space="PSUM") as pspool,
        ):
            pool = ppool.tile([P, OC, NPAIR_eff], ydt)
            out_sb = opool.tile([P, OC, BT_eff], ydt)
            emitted = [False] * NSTORE

            def emit_stores(done_cols):
                for st in range(NSTORE):
                    if not emitted[st] and done_cols >= bound[st]:
                        lo = bound[st - 1] if st else 0
                        sl = slice(lo, bound[st])
                        nc.sync.dma_start(
                            out=out_d[:, :, sl], in_=out_sb[:, :, sl]
                        )
                        emitted[st] = True

            eorder = sched["eorder"]
            for p in range(E):
                e_src = int(eorder[p])
                L, off = int(seg_len[p]), int(seg_off[p])
                # one tile PER ko chunk: Tile deps are tile-granular, so a
                # matmul only waits for the chunk it actually reads
                w_k = [wpool.tile([P, O], dt, tag=f"w{h}", name=f"wk{h}")
                       for h in range(KO)]

                def emit_w(h, eng):
                    eng.dma_start(out=w_k[h][:], in_=w_d[e_src, :, h, :])

                if p == 0:  # small lead chunk: first matmuls start sooner
                    seg_chunks = [(0, 256)] + [
                        (256 + l0, nn_) for l0, nn_ in _chunks(L - 256)
                    ]
                    # ramp-critical emission, balanced 8+8 across BOTH
                    # HWDGE queues (issue ~0.65us/op each): together they
                    # deliver one full (W, xl) ko-pair per 0.65us, in ko
                    # order, which leads the ko-outer sweep's consumption
                    # (one ko-pair per ~0.44us starting only at +10.8)
                    ln = seg_chunks[0][1]
                    xl = [lpool.tile([P, 256], dt, tag=f"xl{h}", name=f"xl{h}")
                          for h in range(KO)]

                    def emit_xl(h, eng):
                        eng.dma_start(
                            out=xl[h][:, :ln], in_=xt_d[:, h, off : off + ln]
                        )

                    for h in range(KO):
                        if h % 2 == 0:
                            emit_w(h, nc.scalar)
                            emit_xl(h, nc.sync)
                        else:
                            emit_w(h, nc.sync)
                            emit_xl(h, nc.scalar)
                else:
                    seg_chunks = _chunks(L)
                    for h in range(KO):
                        emit_w(h, nc.scalar if h % 2 == 0 else nc.sync)
                for ci, (l0, nn_) in enumerate(seg_chunks):
                    xcf = xpool.tile([P, KO, CHMAX], dt, tag="x")
                    xc = xcf[:, :, :nn_]
                    if p == 0 and ci == 0:
                        pass  # lead xc already emitted above (xl tiles)
                    else:
                        nc.sync.dma_start(
                            out=xc, in_=xt_d[:, :, off + l0 : off + l0 + nn_]
                        )
                    use_xl = p == 0 and ci == 0

                    def evict(oc, ps):
                        if oc % 2 == 0 or os.environ.get("MOE_NO_ACT"):
                            nc.vector.tensor_copy(
                                out=pool[:, oc, off + l0 : off + l0 + nn_],
                                in_=ps,
                            )
                        else:
                            nc.scalar.copy(
                                out=pool[:, oc, off + l0 : off + l0 + nn_],
                                in_=ps,
                            )

                    if p == 0:
                        # W arrives chunk-by-chunk at DMA-issue rate: run
                        # ko OUTER over half the oc banks so each arriving
                        # W chunk feeds 4 matmuls (consumption matches
                        # delivery); 4 PSUM banks per half keep eviction
                        # of one half under the other half's matmuls.
                        for half in range(2):
                            pss = []
                            for oc in range(half * 4, half * 4 + 4):
                                psf = pspool.tile(
                                    [P, CHMAX], mybir.dt.float32, tag="ps"
                                )
                                pss.append(psf[:, :nn_])
                            for ko in range(KO):
                                for k, oc in enumerate(
                                    range(half * 4, half * 4 + 4)
                                ):
                                    nc.tensor.matmul(
                                        out=pss[k],
                                        lhsT=w_k[ko][:, oc * P : (oc + 1) * P],
                                        rhs=(xl[ko][:, :nn_]
                                             if use_xl
                                             else xc[:, ko, :]),
                                        start=(ko == 0),
                                        stop=(ko == KO - 1),
                                    )
                            for k, oc in enumerate(range(half * 4, half * 4 + 4)):
                                evict(oc, pss[k])
                    else:
                        for oc in range(OC):
                            psf = pspool.tile(
                                [P, CHMAX], mybir.dt.float32, tag="ps"
                            )
                            ps = psf[:, :nn_]
                            for ko in range(KO):
                                nc.tensor.matmul(
                                    out=ps,
                                    lhsT=w_k[ko][:, oc * P : (oc + 1) * P],
                                    rhs=xc[:, ko, :],
                                    start=(ko == 0),
                                    stop=(ko == KO - 1),
                                )
                            evict(oc, ps)
                if p == E - 1:
                    # final group: oc-half adds, all low-half ops first —
                    # their evictions (oc 0-3, emitted before oc 4-7)
                    # complete while the high-oc matmuls still run, so the
                    # whole low pass hides under them (DVE is FIFO)
                    for h0 in (0, 4):
                        for cb in combos:
                            if cb["ready"] != p:
                                continue
                            nc.vector.tensor_add(
                                out=out_sb[:, h0 : h0 + 4,
                                           cb["o"] : cb["o"] + cb["n"]],
                                in0=pool[:, h0 : h0 + 4,
                                         cb["a"] : cb["a"] + cb["n"]],
                                in1=pool[:, h0 : h0 + 4,
                                         cb["b"] : cb["b"] + cb["n"]],
                            )
                            if h0 == 4:
                                emit_stores(cb["o"] + cb["n"])
                else:
                    for cb in combos:
                        if cb["ready"] == p:
                            nc.vector.tensor_add(
                                out=out_sb[:, :, cb["o"] : cb["o"] + cb["n"]],
                                in0=pool[:, :, cb["a"] : cb["a"] + cb["n"]],
                                in1=pool[:, :, cb["b"] : cb["b"] + cb["n"]],
                            )
                done = sum(cb["n"] for cb in combos if cb["ready"] <= p)
                emit_stores(done)
            assert all(emitted)
            if dbg:
                nc.sync.dma_start(out=pool_d[:, :, :], in_=pool[:])
    return nc


def kernel(x, gates, W, b):
    _patch_tile_drain()
    dt, np_dt = _DT_MAP[os.environ.get("MOE_DT", "float16")]
    ydt, _ = _DT_MAP[os.environ.get("MOE_YDT", "float16")]

    gates = np.asarray(gates)
    x = np.ascontiguousarray(x)
    W = np.asarray(W)
    b = np.asarray(b)
    assert not np.any(b != 0)

    sched, fills = _plan(gates)
    nc = _build_program(sched, dt, ydt)
    wb = W.astype(np_dt).reshape(E, KO, P, O).transpose(0, 2, 1, 3).copy()
    in_maps = []
    for c in range(N_CORES):
        im = _core_inputs(x, sched, fills[c], np_dt)
        im["w"] = wb
        in_maps.append(im)

    trace = os.environ.get("MOE_TRACE", "0") == "1"
    kwargs = {}
    if trace:
        _install_ntff_shim()
        kwargs = dict(trace=True, trace_cores=list(range(N_CORES)))

    res = bass_utils.run_bass_kernel_spmd(
        nc, in_maps, core_ids=list(range(N_CORES)), **kwargs
    )
    if trace and res.exec_time_ns is not None:
        print(f"HW exec time: {res.exec_time_ns} ns "
              f"(single NEFF; mean {res.mean_exec_time_ns:.0f})")
    out = np.empty((B, O), np.float32)
    for c in range(N_CORES):
        ot = res.results[c]["out"]  # [P, OC, BT_eff]
        rows = ot.transpose(2, 1, 0).reshape(sched["BT_eff"], O)
        tok = fills[c]["tok"]
        real = tok >= 0
        out[tok[real]] = rows[real].astype(np.float32)
    return out
